# revision 1
# baseline (speedup 1.0000x reference)
"""Trainium2 Bass kernel for a DehazeBlock:
    res1 = relu(conv3x3(x, w1) + b1) + x
    res2 = conv3x3(res1, w2) + b2
    out  = deform_conv(res2, p_w, p_b, dw) + x

Sharding: 8 cores = 4 batch x 2 H-halves (32 rows each, data-parallel,
communication-free; each core gets a zero-padded 40-row input slab).

Deformable conv strategy (all-PE, no gathers): since offsets |t| < 1, a
bilinear sample at (base + t) decomposes over a 3-tap stencil with weights
(relu(-t), 1-|t|, relu(t)).  Folding the per-tap 256x256 channel mix first
(y_n = DW_n @ res2 over all padded pixels), the output becomes, per tap, a
banded matmul  out[m,u] += sum_v y_n[v,m] * B_n[v,u]  where B_n has 9
diagonals holding the per-pixel stencil weights.  B_n is built on-chip with
iota + local_scatter from weight planes assembled by shifted-DMA reads of a
skew-stored DRAM staging layout.
"""

import os
import numpy as np
import ml_dtypes

import bass_rust
import concourse.bass as bass
import concourse.mybir as mybir
import concourse.tile as tile
from concourse import bacc
from concourse.bass_utils import run_bass_kernel_spmd
from concourse.masks import make_identity

bf16 = ml_dtypes.bfloat16
F32 = mybir.dt.float32
BF = mybir.dt.bfloat16
I16 = mybir.dt.int16

P = 128
CB = 2              # channel blocks (256 = 2*128)
W = 66              # padded row width
TS = 40             # x slab rows
R1 = 38             # res1 rows
R2 = 36             # res2 rows (= v rows)
RO = 32             # output rows per core
U = RO * W          # 2112 output pixel space
XL = TS * W + 2     # 2642 padded flat x row-span (+1 lead, +1 tail elem)
R1L = R1 * W + 2    # 2510
VT = 19             # v tiles
VPW = VT * P        # 2432
FW = 2720           # staging row width for F/G planes
MARG = 266
BW = 262            # banded-matrix u-window width per (tap, vtile)
TAPS = 9
N_CORES = 8

CONV1_CHUNKS = [(0, 7), (7, 7), (14, 7), (21, 7), (28, 7), (35, 3)]
CONV2_CHUNKS = [(0, 6), (6, 6), (12, 6), (18, 6), (24, 6), (30, 6)]
OFFS_CHUNKS = [(0, 7), (7, 7), (14, 7), (21, 7), (28, 4)]
UBLOCKS = [(0, 11), (11, 11), (22, 10)]  # (row start, rows)

_CACHE = {}
LAST_RESULTS = None


def _mk_src(t, dims, off):
    s = t.ap().copy()
    s.ap = bass_rust.VecI64Pair(dims)
    s.offset = off
    return s


def _split_at_banks(lo, hi, base):
    """Split [lo, hi) (psum-tile-relative) at 512-elem bank boundaries."""
    segs = []
    a = lo
    while a < hi:
        b = min(hi, ((a - base) // 512 + 1) * 512 + base)
        segs.append((a, b))
        a = b
    return segs


def _build_program():
    nc = bacc.Bacc("TRN2", target_bir_lowering=False, debug=False,
                   num_devices=N_CORES)

    # ---------------- dram I/O ----------------
    xs_d = nc.dram_tensor("xs", [CB, P, XL], F32, kind="ExternalInput")
    xsb_d = nc.dram_tensor("xsb", [CB, P, XL], BF, kind="ExternalInput")
    maskr_d = nc.dram_tensor("maskr", [P, TS], BF, kind="ExternalInput")
    w1t_d = nc.dram_tensor("w1t", [CB, TAPS, P, 256], BF, kind="ExternalInput")
    w2t_d = nc.dram_tensor("w2t", [CB, TAPS, P, 256], BF, kind="ExternalInput")
    pwt_d = nc.dram_tensor("pwt", [CB, TAPS, P, 18], BF, kind="ExternalInput")
    dwt_d = nc.dram_tensor("dwt", [CB, P, TAPS * 256], BF, kind="ExternalInput")
    b1_d = nc.dram_tensor("b1", [CB, P, 1], F32, kind="ExternalInput")
    b2_d = nc.dram_tensor("b2", [CB, P, 1], F32, kind="ExternalInput")
    pb_d = nc.dram_tensor("pb", [18, 1], F32, kind="ExternalInput")
    out_d = nc.dram_tensor("out", [CB, P, RO, 64], F32, kind="ExternalOutput")

    fdram = nc.dram_tensor("fdram", [27, FW], BF)
    gdram = nc.dram_tensor("gdram", [36, FW], BF)

    RELU = mybir.ActivationFunctionType.Relu
    ABS = mybir.ActivationFunctionType.Abs
    IDENT = mybir.ActivationFunctionType.Identity
    MUL = mybir.AluOpType.mult
    ADD = mybir.AluOpType.add

    with tile.TileContext(nc) as tc:
        with tc.tile_pool(name="perm", bufs=1) as perm, \
             tc.tile_pool(name="chunk", bufs=3) as chunk:

            # ---------------- permanent residents ----------------
            xs = [perm.tile([P, XL], F32, name=f"xs{c}") for c in range(CB)]
            dwt = [perm.tile([P, TAPS * 256], BF, name=f"dwt{c}") for c in range(CB)]
            res2b = [perm.tile([P, VPW], BF, name=f"res2b{c}") for c in range(CB)]
            sw = perm.tile([108, VPW], BF, name="sw")
            ident = perm.tile([P, P], BF, name="ident")
            iotaA = perm.tile([P, 60], I16, name="iotaA")
            iotaB = perm.tile([P, 48], I16, name="iotaB")
            maskr = perm.tile([P, TS], BF, name="maskr")
            b1 = [perm.tile([P, 1], F32, name=f"b1{c}") for c in range(CB)]
            b2 = [perm.tile([P, 1], F32, name=f"b2{c}") for c in range(CB)]
            pb = perm.tile([18, 1], F32, name="pb")

            for c in range(CB):
                nc.sync.dma_start(out=xs[c][:], in_=xs_d.ap()[c])
                nc.sync.dma_start(out=dwt[c][:], in_=dwt_d.ap()[c])
                nc.sync.dma_start(out=b1[c][:], in_=b1_d.ap()[c])
                nc.sync.dma_start(out=b2[c][:], in_=b2_d.ap()[c])
            nc.sync.dma_start(out=pb[:], in_=pb_d.ap())
            nc.sync.dma_start(out=maskr[:], in_=maskr_d.ap())

            make_identity(nc, ident[:])
            nc.gpsimd.iota(iotaA[:], pattern=[[BW, 5], [-66, 3], [1, 4]],
                           base=131, channel_multiplier=1)
            nc.gpsimd.iota(iotaB[:], pattern=[[BW, 4], [-66, 3], [1, 4]],
                           base=131, channel_multiplier=1)
            for c in range(CB):
                nc.vector.memset(res2b[c][:, R2 * W:VPW], 0)

            # ============ phases 1-4 (freed before phase 5) ============
            with tc.tile_pool(name="early", bufs=1) as early, \
                 tc.tile_pool(name="scratch", bufs=1) as scratch:
                xsb = [early.tile([P, XL], BF, name=f"xsb{c}") for c in range(CB)]
                w1t = [early.tile([P, TAPS * 256], BF, name=f"w1t{c}") for c in range(CB)]
                w2t = [early.tile([P, TAPS * 256], BF, name=f"w2t{c}") for c in range(CB)]
                pwt = [early.tile([P, TAPS * 18], BF, name=f"pwt{c}") for c in range(CB)]
                res1b = [early.tile([P, R1L], BF, name=f"res1b{c}") for c in range(CB)]
                offs = early.tile([18, U], F32, name="offs")

                for c in range(CB):
                    nc.sync.dma_start(out=xsb[c][:], in_=xsb_d.ap()[c])
                    for t in range(TAPS):
                        nc.sync.dma_start(out=w1t[c][:, t * 256:(t + 1) * 256],
                                          in_=w1t_d.ap()[c, t])
                        nc.sync.dma_start(out=w2t[c][:, t * 256:(t + 1) * 256],
                                          in_=w2t_d.ap()[c, t])
                        nc.sync.dma_start(out=pwt[c][:, t * 18:(t + 1) * 18],
                                          in_=pwt_d.ap()[c, t])
                    nc.vector.memset(res1b[c][:, 0:1], 0)
                    nc.vector.memset(res1b[c][:, R1L - 1:R1L], 0)

                # ---- phase 1: conv1 -> res1b ----
                with tc.tile_pool(name="cpsum", bufs=6, space="PSUM") as cpsum:
                    for mb in range(CB):
                        psums = [cpsum.tile([P, 462], F32, tag="c1ps",
                                            name=f"c1ps_{mb}_{i}")
                                 for i in range(len(CONV1_CHUNKS))]
                        for cb in range(CB):
                            for t in range(TAPS):
                                ky, kx = t // 3, t % 3
                                lhsT = w1t[cb][:, t * 256 + mb * P:
                                               t * 256 + mb * P + P]
                                first = (cb == 0 and t == 0)
                                last = (cb == CB - 1 and t == TAPS - 1)
                                for ci, (c0, cr) in enumerate(CONV1_CHUNKS):
                                    o = 1 + (c0 + ky) * W + kx - 1
                                    nc.tensor.matmul(
                                        psums[ci][:, :cr * W], lhsT,
                                        xsb[cb][:, o:o + cr * W],
                                        start=first, stop=last)
                        for ci, (c0, cr) in enumerate(CONV1_CHUNKS):
                            n = cr * W
                            tmp = chunk.tile([P, 462], F32, tag="post")
                            nc.scalar.activation(tmp[:, :n], psums[ci][:, :n],
                                                 RELU, bias=b1[mb][:], scale=1.0)
                            nc.vector.tensor_add(
                                tmp[:, :n], tmp[:, :n],
                                xs[mb][:, 1 + (c0 + 1) * W: 1 + (c0 + 1) * W + n])
                            mv = maskr[:, c0 + 1:c0 + 1 + cr, None] \
                                .to_broadcast((P, cr, W))
                            ov = res1b[mb][:, 1 + c0 * W: 1 + (c0 + cr) * W]
                            nc.vector.tensor_tensor(
                                ov.rearrange("p (r w) -> p r w", w=W),
                                tmp[:, :n].rearrange("p (r w) -> p r w", w=W),
                                mv, MUL)
                    for c in range(CB):
                        v = res1b[c][:, 1:1 + R1 * W].rearrange(
                            "p (r w) -> p r w", w=W)
                        nc.vector.memset(v[:, :, 0:1], 0)
                        nc.vector.memset(v[:, :, 65:66], 0)

                # ---- phase 2: conv2 -> res2b ----
                with tc.tile_pool(name="c2psum", bufs=6, space="PSUM") as c2psum:
                    for mb in range(CB):
                        psums = [c2psum.tile([P, 396], F32, tag="c2ps",
                                             name=f"c2ps_{mb}_{i}")
                                 for i in range(len(CONV2_CHUNKS))]
                        for cb in range(CB):
                            for t in range(TAPS):
                                ky, kx = t // 3, t % 3
                                lhsT = w2t[cb][:, t * 256 + mb * P:
                                               t * 256 + mb * P + P]
                                first = (cb == 0 and t == 0)
                                last = (cb == CB - 1 and t == TAPS - 1)
                                for ci, (e0, cr) in enumerate(CONV2_CHUNKS):
                                    o = 1 + (e0 + ky) * W + kx - 1
                                    nc.tensor.matmul(
                                        psums[ci][:, :cr * W], lhsT,
                                        res1b[cb][:, o:o + cr * W],
                                        start=first, stop=last)
                        for ci, (e0, cr) in enumerate(CONV2_CHUNKS):
                            n = cr * W
                            tmp = chunk.tile([P, 462], F32, tag="post")
                            nc.scalar.activation(tmp[:, :n], psums[ci][:, :n],
                                                 IDENT, bias=b2[mb][:], scale=1.0)
                            mv = maskr[:, e0 + 2:e0 + 2 + cr, None] \
                                .to_broadcast((P, cr, W))
                            ov = res2b[mb][:, e0 * W:(e0 + cr) * W]
                            nc.vector.tensor_tensor(
                                ov.rearrange("p (r w) -> p r w", w=W),
                                tmp[:, :n].rearrange("p (r w) -> p r w", w=W),
                                mv, MUL)
                    for c in range(CB):
                        v = res2b[c][:, 0:R2 * W].rearrange("p (r w) -> p r w", w=W)
                        nc.vector.memset(v[:, :, 0:1], 0)
                        nc.vector.memset(v[:, :, 65:66], 0)

                # ---- phase 3: offset conv -> offs ----
                with tc.tile_pool(name="opsum", bufs=5, space="PSUM") as opsum:
                    psums = [opsum.tile([18, 462], F32, tag="ops",
                                        name=f"ops_{i}")
                             for i in range(len(OFFS_CHUNKS))]
                    for cb in range(CB):
                        for t in range(TAPS):
                            ky, kx = t // 3, t % 3
                            lhsT = pwt[cb][:, t * 18:(t + 1) * 18]
                            first = (cb == 0 and t == 0)
                            last = (cb == CB - 1 and t == TAPS - 1)
                            for ci, (i0, cr) in enumerate(OFFS_CHUNKS):
                                o = (i0 + 1 + ky) * W + kx - 1
                                nc.tensor.matmul(
                                    psums[ci][:, :cr * W], lhsT,
                                    res2b[cb][:, o:o + cr * W],
                                    start=first, stop=last)
                    for ci, (i0, cr) in enumerate(OFFS_CHUNKS):
                        n = cr * W
                        nc.scalar.activation(offs[:, i0 * W:i0 * W + n],
                                             psums[ci][:, :n], IDENT,
                                             bias=pb[:], scale=1.0)

                # ---- phase 4: F/G planes + staging round trip ----
                # off_c must start at partition 0 for engine ops: move via DMA
                offc = scratch.tile([9, U], F32, tag="s4")
                nc.sync.dma_start(out=offc[:], in_=offs[9:18, :])
                fsb = scratch.tile([96, U], BF, tag="s1")
                gsb = scratch.tile([96, U], BF, tag="s2")
                tmpa = scratch.tile([9, U], F32, tag="s3")
                nc.scalar.activation(fsb[0:9, :], offs[0:9, :], RELU, scale=-1.0)
                nc.scalar.activation(tmpa[0:9, :], offs[0:9, :], ABS)
                nc.vector.tensor_scalar(fsb[32:41, :], tmpa[0:9, :], -1.0, 1.0,
                                        MUL, ADD)
                nc.scalar.activation(fsb[64:73, :], offs[0:9, :], RELU, scale=1.0)
                tmpb = scratch.tile([9, U], F32, tag="s5")
                nc.scalar.activation(gsb[0:9, :], offc[:], RELU, scale=-1.0)
                nc.scalar.activation(tmpb[0:9, :], offc[:], ABS)
                nc.vector.tensor_scalar(gsb[32:41, :], tmpb[0:9, :], -1.0, 1.0,
                                        MUL, ADD)
                nc.scalar.activation(gsb[64:73, :], offc[:], RELU, scale=1.0)

                zsb = scratch.tile([36, FW], BF, tag="s1z")
                nc.vector.memset(zsb[:], 0)
                nc.sync.dma_start(out=fdram.ap(), in_=zsb[:27, :])
                nc.sync.dma_start(out=gdram.ap(), in_=zsb[:, :])
                # skew-store F: f_{dr}(off_r[n])(u) at
                #   fdram[3n+i_dr, MARG + 66*i_nr + i_nc + 66*i_dr + u]
                for i_dr in range(3):
                    for i_nr in range(3):
                        dst = _mk_src(
                            fdram, [[3 * FW + 1, 3], [1, U]],
                            (9 * i_nr + i_dr) * FW + MARG + 66 * i_nr + 66 * i_dr)
                        nc.sync.dma_start(
                            out=dst,
                            in_=fsb[32 * i_dr + 3 * i_nr: 32 * i_dr + 3 * i_nr + 3, :])
                # skew-store G: g_{dc}(off_c[n])(u) at
                #   gdram[4n+i_dc, MARG + 66*i_nr + i_nc + i_dc + u]
                for jc, grow in [(1, 64), (2, 32), (3, 0)]:
                    for i_nr in range(3):
                        dst = _mk_src(
                            gdram, [[4 * FW + 1, 3], [1, U]],
                            (12 * i_nr + jc) * FW + MARG + 66 * i_nr + (3 - jc))
                        nc.sync.dma_start(
                            out=dst,
                            in_=gsb[grow + 3 * i_nr: grow + 3 * i_nr + 3, :])
                # shifted reads -> P_r, P_c in slot order l = 12n + 4*i_dr + i_dc
                prt = scratch.tile([108, VPW], BF, tag="s2p")
                pct = scratch.tile([108, VPW], BF, tag="s3p")
                for n in range(TAPS):
                    src = _mk_src(fdram, [[FW, 3], [1, 4], [1, VPW]],
                                  3 * n * FW + MARG - 1)
                    nc.sync.dma_start(out=prt[12 * n:12 * n + 12, :], in_=src)
                    for i_dr in range(3):
                        src = _mk_src(gdram, [[FW, 4], [1, VPW]],
                                      4 * n * FW + MARG + 2 - 66 * i_dr)
                        nc.sync.dma_start(
                            out=pct[12 * n + 4 * i_dr:12 * n + 4 * i_dr + 4, :],
                            in_=src)
                nc.vector.tensor_tensor(sw[:], prt[:], pct[:], MUL)

            # ============ phase 5: deform banded matmuls ============
            with tc.tile_pool(name="ypool", bufs=11) as ypool, \
                 tc.tile_pool(name="swtpool", bufs=11) as swtpool, \
                 tc.tile_pool(name="b5apool", bufs=11) as b5apool, \
                 tc.tile_pool(name="b5bpool", bufs=11) as b5bpool, \
                 tc.tile_pool(name="ypsum", bufs=1, space="PSUM") as ypsum, \
                 tc.tile_pool(name="bpsum", bufs=1, space="PSUM") as bpsum, \
                 tc.tile_pool(name="tpsum", bufs=1, space="PSUM") as tpsum:

                for (ob, rb) in UBLOCKS:
                    U0, UW = ob * W, rb * W
                    built = {}
                    mms = []
                    for vt in range(VT):
                        v0 = vt * P
                        hit = False
                        for n in range(TAPS):
                            nr, ncc = n // 3 - 1, n % 3 - 1
                            w0 = v0 - 199 - 66 * nr - ncc
                            lo, hi = max(w0, U0), min(w0 + BW, U0 + UW)
                            if lo >= hi:
                                continue
                            hit = True
                            mms.append((vt, n, w0, lo, hi))
                        if hit and vt not in built:
                            # build y / swT / B5 for this vtile
                            psy = ypsum.tile([P, TAPS * 256], F32, tag="psy")
                            for cb in range(CB):
                                for pc in range(5):
                                    a = pc * 512
                                    b = min(a + 512, TAPS * 256)
                                    nc.tensor.matmul(psy[:, a:b],
                                                     res2b[cb][:, v0:v0 + P],
                                                     dwt[cb][:, a:b],
                                                     start=(cb == 0),
                                                     stop=(cb == CB - 1))
                            y = ypool.tile([P, TAPS * 256], BF, tag="y")
                            nc.vector.tensor_copy(y[:], psy[:])
                            pst = tpsum.tile([P, P], BF, tag="pst")
                            nc.tensor.transpose(pst[:, :108], sw[:, v0:v0 + P],
                                                ident[:108, :108])
                            swT = swtpool.tile([P, 108], BF, tag="swT")
                            nc.vector.tensor_copy(swT[:], pst[:, :108])
                            b5a = b5apool.tile([P, 5 * BW], BF, tag="b5a")
                            b5b = b5bpool.tile([P, 4 * BW], BF, tag="b5b")
                            nc.gpsimd.local_scatter(b5a[:], swT[:, 0:60],
                                                    iotaA[:], channels=P,
                                                    num_elems=5 * BW, num_idxs=60)
                            nc.gpsimd.local_scatter(b5b[:], swT[:, 60:108],
                                                    iotaB[:], channels=P,
                                                    num_elems=4 * BW, num_idxs=48)
                            built[vt] = (y, b5a, b5b)

                    for mb in range(CB):
                        psb = bpsum.tile([P, UW], F32, tag="psb")
                        nc.vector.memset(psb[:], 0)
                        for j, (vt, n, w0, lo, hi) in enumerate(mms):
                            y, b5a, b5b = built[vt]
                            lhsT = y[:, n * 256 + mb * P: n * 256 + mb * P + P]
                            for (sa, sb_) in _split_at_banks(lo, hi, U0):
                                if n < 5:
                                    rhs = b5a[:, n * BW + sa - w0:
                                              n * BW + sb_ - w0]
                                else:
                                    rhs = b5b[:, (n - 5) * BW + sa - w0:
                                              (n - 5) * BW + sb_ - w0]
                                nc.tensor.matmul(psb[:, sa - U0:sb_ - U0],
                                                 lhsT, rhs, start=False,
                                                 stop=(j == len(mms) - 1),
                                                 skip_group_check=True)
                        outt = chunk.tile([P, 726], F32, tag="outstage")
                        nc.vector.tensor_add(
                            outt[:, :UW], psb[:],
                            xs[mb][:, 1 + (ob + 4) * W: 1 + (ob + 4) * W + UW])
                        nc.sync.dma_start(
                            out=out_d.ap()[mb, :, ob:ob + rb, :],
                            in_=outt[:, :UW].rearrange(
                                "p (r w) -> p r w", w=W)[:, :, 1:65])

    nc.finalize()
    return nc


def _pack_inputs(x, w1, b1, w2, b2, p_w, p_b, dw):
    """Build the 8 per-core input maps (numpy only)."""
    x = np.asarray(x, np.float32)

    def pack_w(w, mout):
        w = np.asarray(w, np.float32)
        out = np.empty((CB, TAPS, P, mout), bf16)
        for cb in range(CB):
            for t in range(TAPS):
                out[cb, t] = w[:, cb * P:(cb + 1) * P, t // 3, t % 3].T.astype(bf16)
        return out

    w1t = pack_w(w1, 256)
    w2t = pack_w(w2, 256)
    pwt = pack_w(p_w, 18)
    dwt = np.empty((CB, P, TAPS * 256), bf16)
    dwf = np.asarray(dw, np.float32)
    for cb in range(CB):
        for t in range(TAPS):
            dwt[cb, :, t * 256:(t + 1) * 256] = \
                dwf[:, cb * P:(cb + 1) * P, t // 3, t % 3].T.astype(bf16)
    b1p = np.ascontiguousarray(np.asarray(b1, np.float32).reshape(CB, P, 1))
    b2p = np.ascontiguousarray(np.asarray(b2, np.float32).reshape(CB, P, 1))
    pbp = np.ascontiguousarray(np.asarray(p_b, np.float32).reshape(18, 1))

    maps = []
    for core in range(N_CORES):
        b, half = core // 2, core % 2
        r0 = 32 * half
        slab = np.zeros((CB, P, TS, W), np.float32)
        g0, g1 = max(0, r0 - 4), min(64, r0 + 36)
        t0 = g0 - (r0 - 4)
        for cb in range(CB):
            slab[cb, :, t0:t0 + (g1 - g0), 1:65] = \
                x[b, cb * P:(cb + 1) * P, g0:g1, :]
        xsv = np.zeros((CB, P, XL), np.float32)
        xsv[:, :, 1:1 + TS * W] = slab.reshape(CB, P, TS * W)
        maskr = np.zeros((P, TS), bf16)
        valid = np.array([1.0 if 0 <= r0 - 4 + t < 64 else 0.0
                          for t in range(TS)], np.float32)
        maskr[:] = valid.astype(bf16)[None, :]
        maps.append({
            "xs": xsv, "xsb": xsv.astype(bf16), "maskr": maskr,
            "w1t": w1t, "w2t": w2t, "pwt": pwt, "dwt": dwt,
            "b1": b1p, "b2": b2p, "pb": pbp,
        })
    return maps


def get_program():
    if "nc" not in _CACHE:
        _CACHE["nc"] = _build_program()
    return _CACHE["nc"]


def _ensure_ntff_hook():
    """The image's antenv lacks axon_hooks; inject a shim and register the
    NTFF profiling hook so trace=True works under axon."""
    import sys, types
    import antenv
    if "antenv.axon_hooks" in sys.modules:
        return
    mod = types.ModuleType("antenv.axon_hooks")
    mod._hook = None
    def set_axon_ntff_profile_hook(h):
        mod._hook = h
    def get_axon_ntff_profile_hook():
        return mod._hook
    mod.set_axon_ntff_profile_hook = set_axon_ntff_profile_hook
    mod.get_axon_ntff_profile_hook = get_axon_ntff_profile_hook
    sys.modules["antenv.axon_hooks"] = mod
    antenv.axon_hooks = mod
    try:
        from trn_agent_boot.trn_boot import _ntff_profile_via_ctypes
        hook = _ntff_profile_via_ctypes("/opt/axon/libaxon_pjrt.so")
        if hook is not None:
            set_axon_ntff_profile_hook(hook)
    except Exception as e:
        print("ntff hook setup failed:", e)


def kernel(x, w1, b1, w2, b2, p_w, p_b, dw):
    global LAST_RESULTS
    nc = get_program()
    maps = _pack_inputs(x, w1, b1, w2, b2, p_w, p_b, dw)
    trace = os.environ.get("DEHAZE_TRACE") == "1"
    if trace:
        _ensure_ntff_hook()
    res = run_bass_kernel_spmd(nc, maps, core_ids=list(range(N_CORES)),
                               trace=trace)
    LAST_RESULTS = res
    out = np.empty((4, 256, 64, 64), np.float32)
    for core in range(N_CORES):
        b, half = core // 2, core % 2
        o = res.results[core]["out"]  # [CB, P, RO, 64]
        out[b, :, 32 * half:32 * half + 32, :] = o.reshape(256, 32, 64)
    return out

